# revision 44
# baseline (speedup 1.0000x reference)
"""Trainium2 Bass kernel for CRATE-style subspace attention (nn_Attention_37091337568712).

Reference computation (fp32):
    w = x @ Wqkv                    # (b, n, 1024), shared q=k=v projection
    w -> (b, h=16, n, d=64)
    S = (w @ w^T) * d^-0.5          # per head, (b, h, n, n)
    attn = softmax(S, axis=-1) * (1 - mask[:, None, None, :])
    out = attn @ w                  # (b, h, n, d)
    y = out.reshape(b, n, 1024) @ Wout + bout

Sharding: 8 cores = 2 batches x 4 head-groups (4 heads each). Each core
computes its 4 heads end-to-end including a partial output projection
(Wout rows for its heads); host sums the 4 partials per batch (the
"all-reduce" of the output projection) and adds bout.

Device kernel (per core) highlights (~243 us HW, vs 397 us f32r baseline):
  - all matmul operands bf16 (fp32 PSUM accumulation): ~3x the per-column
    PE throughput of the f32r fp32_mode=HIGH 3-pass path
  - exp split across two engines: ScalarE runs the real exp (table
    spline); for half the key chunks the DVE computes the bf16 BIT
    PATTERN of exp directly via one int16 tensor_scalar (Schraudolph:
    i16 = S*(scale*2^7/ln2) + 127*2^7 - C, bitcast to bf16; ~3% sawtooth
    that averages out in the softmax, measured 6.8e-3 final rel err)
  - single-head sweeps with a 3-deep S-tile PSUM pipeline keep the PE
    ~saturated so the HAM clock-gate holds 2.4 GHz through phase 3;
    jc pairs ([S,S][expA||expV][AV,AV]) halve LDWEIGHTS row-group bubbles
  - softmax denominator comes free: V' carries an unmasked ones column so
    row 64 of the AV accumulator is the denominator
  - post-softmax column mask pre-folded into wmT2 = (1-mask_i)*wT2 (PE
    ones-matmul broadcast of the mask row), V' via PE transposes of wmT2
    with one strided [128,2,64] copy each
  - the scale chain (den spread-DMA -> reciprocal -> row gather ->
    partition broadcast -> osT2 multiply) is a 4-sweep-deep software
    pipeline: every op enters its engine FIFO only after its inputs are
    sweeps old, so the strict in-order engine queues never head-of-line
    block the attention (B1 +2, B2 +3, C +4 sweeps)
  - output projection rides inside the last two sweeps (ibh0 half) and
    the ACT/DVE-shared tail (ibh1 half)
"""

import sys

if "/opt/trn_rl_repo" not in sys.path:
    sys.path.insert(0, "/opt/trn_rl_repo")

import numpy as np

import concourse.bass as bass
import concourse.mybir as mybir
from concourse import masks
from concourse.bass_utils import run_bass_kernel_spmd
from concourse.tile import TileContext

FP = mybir.dt.float32
BF = mybir.dt.bfloat16
I32 = mybir.dt.int32


def _split_multiwaits(bir_json: bytes) -> bytes:
    """This container's walrus supports a single sync wait per instruction
    (setupSyncWait: 'Too many sync wait commands', seen on the Tile tail
    Drain). Split any multi-wait instruction into a chain of single-wait
    EventSemaphore instructions (same engine, program order) followed by
    the original instruction keeping its last wait."""
    import json

    bir = json.loads(bir_json)
    changed = False
    for fn in bir.get("functions", []):
        for bb in fn.get("blocks", []):
            insts = bb.get("instructions")
            if insts is None:
                continue
            new_insts = []
            for ins in insts:
                si = ins.get("sync_info")
                waits = si.get("on_wait") if si else None
                if waits and len(waits) > 1:
                    changed = True
                    for wi, w in enumerate(waits[:-1]):
                        new_insts.append({
                            "name": f"{ins['name']}_w{wi}",
                            "opcode": "EventSemaphore",
                            "engine": ins["engine"],
                            "ins": [],
                            "outs": [],
                            "debug": ins.get("debug", 0),
                            "sync_info": {"on_wait": [w], "on_update": []},
                        })
                    si["on_wait"] = [waits[-1]]
                new_insts.append(ins)
            bb["instructions"] = new_insts
    if not changed:
        return bir_json
    return json.dumps(bir).encode()


def _install_bir_legalizer():
    from concourse import bass2jax, bass_utils

    if getattr(bass2jax, "_multiwait_legalizer_installed", False):
        return
    orig = bass_utils.compile_bir_kernel

    def wrapped(bir_json, tmpdir, neff_name="file.neff"):
        try:
            return orig(_split_multiwaits(bytes(bir_json)), tmpdir, neff_name)
        except BaseException as e:
            # XLA swallows python exceptions from the compile callback;
            # persist the real error for debugging.
            import subprocess, traceback
            try:
                with open("/tmp/bass_compile_err.txt", "w") as f:
                    traceback.print_exc(file=f)
                    ee = e
                    while ee is not None:
                        if isinstance(ee, subprocess.CalledProcessError):
                            out = ee.stdout or ""
                            if isinstance(out, bytes):
                                out = out.decode(errors="replace")
                            f.write("\n==WALRUS STDOUT (tail)==\n" + out[-12000:])
                        ee = ee.__cause__ or ee.__context__
            except Exception:
                pass
            raise

    bass2jax.compile_bir_kernel = wrapped
    bass2jax._multiwait_legalizer_installed = True

N = 2048          # sequence length
DIM = 1024        # model dim
DH = 64           # head dim
HEADS_PER_CORE = 4
PAIRS = 2         # head pairs per core (2 heads = 128 partitions stacked)
EC = HEADS_PER_CORE * DH   # 256 local inner columns
KC = DIM // 128   # 8 contraction chunks for the projection
JC = N // 128     # 16 key chunks
SCALE = DH ** -0.5

# int16-Schraudolph exp approximation (DVE offload): for a subset of key
# chunks, exp(S*SCALE) is computed as bitcast_bf16(int16(S*A + B)) in a
# single DVE tensor_scalar — i.e. build the bf16 bit pattern directly.
# A = SCALE * 2^7/ln2; B = 127*2^7 - C with C~5 tuned for minimax relative
# error (~3% sawtooth; washes out to <1e-2 in the softmax average).
SCHRAUD_A = SCALE * (2.0 ** 7) / np.log(2.0)
SCHRAUD_B = 127.0 * 2 ** 7 - 5.0
SCHRAUD_JC = frozenset({1, 3, 5, 7, 9, 11, 13, 15})   # 8 of 16 chunks -> DVE

_program_cache = {}


def build_program():
    nc = bass.Bass()

    xT = nc.declare_dram_parameter("xT", [DIM, N], BF, isOutput=False)
    wqkv = nc.declare_dram_parameter("wqkv", [DIM, EC], BF, isOutput=False)
    wout = nc.declare_dram_parameter("wout", [EC, DIM], BF, isOutput=False)
    mask_d = nc.declare_dram_parameter("mask", [N], I32, isOutput=False)
    y = nc.declare_dram_parameter("y", [N, DIM], FP, isOutput=True)

    EXPF = mybir.ActivationFunctionType.Exp

    with TileContext(nc) as tc:
        with (
            tc.tile_pool(name="const", bufs=1) as constp,
            tc.tile_pool(name="wts", bufs=1) as wts,
            tc.tile_pool(name="persist", bufs=1) as persist,
            tc.tile_pool(name="xin", bufs=8) as xin,
            tc.tile_pool(name="epool", bufs=10) as epool,
            tc.tile_pool(name="bsb", bufs=2) as bsb,
        ):
            # ---- constants / small inputs ----
            ident = constp.tile([128, 128], BF)
            masks.make_identity(nc, ident[:])
            ones128 = constp.tile([1, 128], BF)
            nc.vector.memset(ones128[:], 1.0)

            mask_i = constp.tile([16, 128], I32)
            nc.sync.dma_start(mask_i[:], mask_d.rearrange("(a b) -> a b", a=16))
            mask_f = constp.tile([16, 128], BF)
            # 1 - mask, cast int32 -> bf16 (0/1 exact)
            nc.vector.tensor_scalar(
                out=mask_f[:], in0=mask_i[:], scalar1=-1.0, scalar2=1.0,
                op0=mybir.AluOpType.mult, op1=mybir.AluOpType.add,
            )
            # (1-mask) as a single [1, N] row (partition-major read of mask_f)
            maskrow = constp.tile([1, N], BF)
            nc.sync.dma_start(maskrow[:], mask_f[:])

            # ---- weights ----
            wq_sb = wts.tile([128, KC, EC], BF)
            nc.sync.dma_start(wq_sb[:], wqkv.rearrange("(kc p) e -> p kc e", p=128))
            wout_sb = wts.tile([128, PAIRS, DIM], BF)
            nc.sync.dma_start(wout_sb[:], wout.rearrange("(pc p) m -> p pc m", p=128))

            # ---- persistent big tiles ----
            wT2 = persist.tile([128, PAIRS, N], BF)        # [d2, pair, i]
            wmT2 = persist.tile([128, PAIRS, N], BF)       # (1-mask_i) * wT2
            maskb = persist.tile([128, N], BF)             # (1-mask) bcast on parts
            v2 = persist.tile([128, PAIRS, JC, 130], BF)   # [j, pair, jc, d2|ones]
            osT2 = persist.tile([128, PAIRS, N], BF)       # scaled attn out, [e, pair, i]
            # softmax denominators, spread [128, 8 per k] for a cheap batched
            # reciprocal (engine APs may only start at partition 0/32/64/96,
            # and DVE reciprocal costs ~8 cycles per element per lane)
            den_sp = persist.tile([128, 64], FP)
            recip_sp = persist.tile([128, 64], BF)   # bf16: feeds the 1-pass
                                                     # broadcast matmul

            # ---- phase 1: projection  wT2[d2, i] = Wqkv_cols^T @ x^T ----
            with tc.tile_pool(name="ps_proj", bufs=1, space="PSUM") as ps_proj:
                proj_ps = [ps_proj.tile([128, 512], FP, name=f"proj{t}", tag=f"proj{t}")
                           for t in range(8)]
                for kc in range(KC):
                    xt = xin.tile([128, N], BF, name="xt")
                    nc.sync.dma_start(xt[:], xT[kc * 128:(kc + 1) * 128, :])
                    for pair in range(PAIRS):
                        for rb in range(4):
                            nc.tensor.matmul(
                                proj_ps[pair * 4 + rb][:],
                                wq_sb[:, kc, pair * 128:(pair + 1) * 128],
                                xt[:, rb * 512:(rb + 1) * 512],
                                start=(kc == 0), stop=(kc == KC - 1),
                            )
                for pair in range(PAIRS):
                    for rb in range(4):
                        dst = wT2[:, pair, rb * 512:(rb + 1) * 512]
                        if rb % 2 == 0:
                            nc.vector.tensor_copy(dst, proj_ps[pair * 4 + rb][:])
                        else:
                            nc.scalar.copy(dst, proj_ps[pair * 4 + rb][:])

                # (1-mask) broadcast across partitions (K=1 ones matmul into
                # the now-free projection PSUM slots), then the pre-masked
                # wmT2 = wT2 * maskb on DVE (bf16 all-SBUF = 2x mode)
                for rb in range(4):
                    mb_ps = proj_ps[rb]
                    nc.tensor.matmul(
                        mb_ps[:], ones128[:],
                        maskrow[0:1, rb * 512:(rb + 1) * 512],
                        start=True, stop=True,
                    )
                    if rb % 2 == 0:
                        nc.vector.tensor_copy(maskb[:, rb * 512:(rb + 1) * 512], mb_ps[:])
                    else:
                        nc.scalar.copy(maskb[:, rb * 512:(rb + 1) * 512], mb_ps[:])
                for pair in range(PAIRS):
                    for half in range(2):
                        nc.vector.tensor_tensor(
                            out=wmT2[:, pair, half * 1024:(half + 1) * 1024],
                            in0=wT2[:, pair, half * 1024:(half + 1) * 1024],
                            in1=maskb[:, half * 1024:(half + 1) * 1024],
                            op=mybir.AluOpType.mult,
                        )

            # ---- phase 2: V' via PE transposes of the pre-masked wmT2 ----
            # ones columns (64 and 129): the AV matmul's M=65 weight includes
            # them so row 64 of the AV accumulator becomes the (unmasked)
            # softmax denominator for free. One strided copy per block
            # ([128, 2, 64] view, stride 65) replaces the old per-head
            # mask-multiplies.
            nc.vector.memset(v2[:, :, :, 64:130:65], 1.0)
            with tc.tile_pool(name="ps_tr", bufs=2, space="PSUM") as ps_tr:
                for pair in range(PAIRS):
                    for jc in range(JC):
                        tr_ps = ps_tr.tile([128, 128], BF, name="tr", tag="tr")
                        nc.tensor.transpose(
                            tr_ps[:], wmT2[:, pair, jc * 128:(jc + 1) * 128],
                            ident[:])
                        dst = v2[:, pair, jc, 0:130].rearrange(
                            "p (b c) -> p b c", b=2, c=65)[:, :, 0:64]
                        if jc % 2 == 0:
                            nc.vector.tensor_copy(dst, tr_ps[:])
                        else:
                            nc.scalar.copy(dst, tr_ps[:])

            # ---- phase 3 + 4: attention, scale, output projection ----
            # Single-head sweeps (one hh at a time): with bf16 1-pass matmuls
            # the PE->PSUM drain port serializes row-split pairs anyway, and
            # per-head sweeps free PSUM banks for a 3-deep S pipeline so the
            # PE runs dense back-to-back bursts (keeps the HAM clock-gate at
            # 2.4 GHz). ibh-outer so each i-half's scale + output projection
            # overlap the other half's ACT-bound attention.
            with (
                tc.tile_pool(name="ps_s", bufs=3, space="PSUM") as ps_s,
                tc.tile_pool(name="ps_av", bufs=1, space="PSUM") as ps_av,
            ):
                def proj_out(ic, on_act):
                    # y[ic-block] = sum_pair osT2[:, pair, ic]^T @ Wout[pair]
                    y_ps = ps_s.tile([128, 1024], FP, name="yps", tag="s")
                    for nb in range(2):
                        for pair in range(PAIRS):
                            nc.tensor.matmul(
                                y_ps[:, nb * 512:(nb + 1) * 512],
                                osT2[:, pair, ic * 128:(ic + 1) * 128],
                                wout_sb[:, pair, nb * 512:(nb + 1) * 512],
                                start=(pair == 0), stop=(pair == PAIRS - 1),
                            )
                    y_sb = bsb.tile([128, 1024], FP, name="ysb", tag="ysb", bufs=3)
                    # ScalarE has slack (DVE carries the exp offload)
                    if on_act:
                        nc.scalar.copy(y_sb[:], y_ps[:])
                    else:
                        nc.vector.tensor_copy(y_sb[:], y_ps[:])
                    nc.sync.dma_start(y[ic * 128:(ic + 1) * 128, :], y_sb[:])

                # Deferred scale-chain stages. Engine FIFOs are strict
                # in-order, so an op whose waits aren't yet satisfied when it
                # reaches an engine blocks everything behind it (including
                # the next sweep's independent matmuls). Each stage is
                # therefore emitted only once its inputs are several sweeps
                # old:
                #   A  (inline): evict av -> raw65 (rows 0..64 = out, 64 =
                #                den), den spread-DMA from the SBUF copy
                #   B1 (+2 sweeps): reciprocal + rrow gather-DMA
                #   B2 (+3): PE ones-matmul broadcast + park in SBUF
                #   C  (+4): osT2 = raw65 * (1/den)
                stages = {}    # sweep index -> stage state

                def emit_b1(st):
                    k = st['k']
                    with nc.allow_low_precision(
                            reason="1/den in bf16 feeds a bf16 broadcast "
                                   "matmul; ~4e-3 rel, inside the 2e-2 gate"):
                        nc.vector.reciprocal(recip_sp[:, k * 8:(k + 1) * 8],
                                             den_sp[:, k * 8:(k + 1) * 8])
                    rrow = bsb.tile([1, 1024], BF, name="rrow", tag="rrow")
                    nc.sync.dma_start(rrow[:], recip_sp[:, k * 8:(k + 1) * 8])
                    st['rrow'] = rrow

                def emit_b2(st):
                    # b_ps borrows an s-tag slot; called mid-sweep (jc==3)
                    # when the slot of the jc==1 S tile is provably free
                    b_ps = ps_s.tile([128, 1024], FP, name="bps", tag="s")
                    for sb in range(2):
                        nc.tensor.matmul(
                            b_ps[:, sb * 512:(sb + 1) * 512],
                            ones128[:],
                            st['rrow'][0:1, sb * 512:(sb + 1) * 512],
                            start=True, stop=True,
                        )
                    b_sb = bsb.tile([128, 1024], FP, name="bsb", tag="bsb")
                    nc.scalar.copy(b_sb[:], b_ps[:])
                    st['b_sb'] = b_sb

                def emit_c(st):
                    i0 = st['ibh'] * 1024
                    p0 = st['hh'] * 64
                    nc.vector.tensor_tensor(
                        out=osT2[p0:p0 + 64, st['pair'], i0:i0 + 1024],
                        in0=st['raw65'][0:64, :],
                        in1=st['b_sb'][0:64, :],
                        op=mybir.AluOpType.mult,
                    )

                def flush_head(sw):
                    # PE-free stages at the head of sweep `sw`
                    if sw - 2 in stages and 'rrow' not in stages[sw - 2]:
                        emit_b1(stages[sw - 2])
                    if sw - 4 in stages:
                        emit_c(stages.pop(sw - 4))

                def flush_mid(sw):
                    # the PE broadcast, a few jc into sweep `sw`
                    if sw - 3 in stages and 'b_sb' not in stages[sw - 3]:
                        emit_b2(stages[sw - 3])

                def flush_all():
                    for sw in sorted(stages):
                        if 'rrow' not in stages[sw]:
                            emit_b1(stages[sw])
                    for sw in sorted(stages):
                        if 'b_sb' not in stages[sw]:
                            emit_b2(stages[sw])
                    for sw in sorted(stages):
                        emit_c(stages[sw])
                    stages.clear()

                def sweep(sw, pair, ibh, hh, ph4_ics=()):
                    i0 = ibh * 1024
                    p0 = hh * 64
                    k = ((ibh * PAIRS + pair) * 2 + hh)
                    av = ps_av.tile([65, 1024], FP, name="av", tag="av")
                    ph4_q = list(ph4_ics)

                    def s_mm(jc, s_t):
                        for sb in range(2):
                            nc.tensor.matmul(
                                s_t[:, sb * 512:(sb + 1) * 512],
                                wT2[p0:p0 + 64, pair, jc * 128:(jc + 1) * 128],
                                wT2[p0:p0 + 64, pair,
                                    i0 + sb * 512:i0 + (sb + 1) * 512],
                                start=True, stop=True,
                                tile_position=(p0, 0),
                            )

                    def exp_tile(jc, s_t):
                        if jc in SCHRAUD_JC:
                            # DVE offload: bf16 bit pattern of exp via int16
                            # affine (see SCHRAUD_* above)
                            e_i = epool.tile([128, 1024], mybir.dt.int16,
                                             name="ei", tag="e")
                            nc.vector.tensor_scalar(
                                out=e_i[:], in0=s_t[:],
                                scalar1=float(SCHRAUD_A), scalar2=float(SCHRAUD_B),
                                op0=mybir.AluOpType.mult, op1=mybir.AluOpType.add,
                            )
                            return e_i[:].bitcast(BF)
                        e = epool.tile([128, 1024], BF, name="e", tag="e")
                        nc.scalar.activation(e[:], s_t[:], EXPF, scale=SCALE)
                        return e[:]

                    def av_mm(jc, e_ap):
                        for sb in range(2):
                            nc.tensor.matmul(
                                av[:, sb * 512:(sb + 1) * 512],
                                v2[:, pair, jc, hh * 65:hh * 65 + 65],
                                e_ap[:, sb * 512:(sb + 1) * 512],
                                start=(jc == 0), stop=(jc == JC - 1),
                                skip_group_check=True,
                            )

                    # jc processed in pairs (one ACT-exp + one DVE-exp each):
                    # [S,S] [exp||exp] [AV,AV] halves the PE weight-switch
                    # (LDWEIGHTS row-group conflict) bubbles
                    for jcp in range(JC // 2):
                        jc0, jc1 = 2 * jcp, 2 * jcp + 1
                        if jc0 == 4:
                            flush_mid(sw)
                        if jc0 >= 2 and ph4_q:
                            w = ph4_q.pop(0)
                            if callable(w):
                                w()
                            else:
                                proj_out(w, on_act=(jcp % 2 == 1))
                        st0 = ps_s.tile([128, 1024], FP, name="s", tag="s")
                        s_mm(jc0, st0)
                        st1 = ps_s.tile([128, 1024], FP, name="s", tag="s")
                        s_mm(jc1, st1)
                        e0 = exp_tile(jc0, st0)
                        e1 = exp_tile(jc1, st1)
                        av_mm(jc0, e0)
                        av_mm(jc1, e1)
                    while ph4_q:
                        w = ph4_q.pop(0)
                        if callable(w):
                            w()
                        else:
                            proj_out(w, on_act=True)
                    # stage A: single eviction copy (65 rows: out + den row);
                    # frees the av slot fast. ScalarE has the slack.
                    raw65 = bsb.tile([65, 1024], FP, name="raw65", tag="raw65",
                                     bufs=5)
                    nc.scalar.copy(raw65[:], av[:])
                    nc.sync.dma_start(den_sp[:, k * 8:(k + 1) * 8],
                                      raw65[64:65, :])
                    stages[sw] = {'pair': pair, 'ibh': ibh, 'hh': hh,
                                  'k': k, 'raw65': raw65}

                sweeps = [(pair, ibh, hh)
                          for ibh in range(2)
                          for pair in range(PAIRS)
                          for hh in range(2)]
                def adv(stage, key):
                    # advance a flush stage into sweep-7 side work (its
                    # inputs are sweeps old by then: waits pre-satisfied)
                    def go():
                        if key in stages:
                            if stage == 'b1' and 'rrow' not in stages[key]:
                                emit_b1(stages[key])
                            elif stage == 'b2' and 'b_sb' not in stages[key]:
                                emit_b2(stages[key])
                            elif stage == 'c' and 'b_sb' in stages[key]:
                                emit_c(stages.pop(key))
                    return go

                for sw, (pair, ibh, hh) in enumerate(sweeps):
                    flush_head(sw)
                    # ibh0's osT2 is complete at the head of sweep 7
                    # (stage C of sweeps 0..3 flushed); its projection
                    # chunks interleave into the last sweep along with
                    # advanced flush stages of completed sweeps
                    if sw == 7:
                        work = [0, 1, 2, adv('b1', 6), 3, adv('b2', 5),
                                4, 5, adv('c', 4), 6, 7, adv('b2', 6),
                                adv('c', 5)]
                    else:
                        work = ()
                    sweep(sw, pair, ibh, hh, ph4_ics=work)
                flush_all()
                # ibh1's output projection runs in the ACT/DVE-shared tail
                for ic in range(8, 16):
                    proj_out(ic, on_act='split')

    return nc


def get_program():
    if "nc" not in _program_cache:
        _program_cache["nc"] = build_program()
    return _program_cache["nc"]


def make_in_maps(x, mask, Wqkv, Wout):
    import ml_dtypes
    bf16 = ml_dtypes.bfloat16
    xT_b = [np.ascontiguousarray(x[b].T).astype(bf16) for b in range(2)]
    wq_bf = Wqkv.astype(bf16)
    wo_bf = Wout.astype(bf16)
    in_maps = []
    for c in range(8):
        b, hg = c // 4, c % 4
        ec = slice(hg * EC, (hg + 1) * EC)
        in_maps.append({
            "xT": xT_b[b],
            "wqkv": np.ascontiguousarray(wq_bf[:, ec]),
            "wout": np.ascontiguousarray(wo_bf[ec, :]),
            "mask": np.ascontiguousarray(mask[b]),
        })
    return in_maps


def assemble(results, bout):
    y = np.stack([
        sum(results[b * 4 + g]["y"] for g in range(4)) for b in range(2)
    ])
    return (y + bout[None, None, :]).astype(np.float32)


def kernel(x, mask, Wqkv, Wout, bout):
    _install_bir_legalizer()
    nc = get_program()
    in_maps = make_in_maps(x, mask, Wqkv, Wout)
    res = run_bass_kernel_spmd(nc, in_maps, core_ids=list(range(8)))
    return assemble(res.results, bout)


if __name__ == "__main__":
    nc = build_program()
    print("program built OK")
